# revision 26
# baseline (speedup 1.0000x reference)
"""AttentiveItemToVec Trainium2 kernel (8 NeuronCores, batch-parallel).

Strategy: fold every id-dependent quantity into host-precomputed lookup
tables so the device kernel is pure gather + attention:

  CT8[v]  = [ 64*T2[v] (0:128) | 64.0 (128) | chat[v] (129:189) | 0 ]
            fp8e4m3, 256 elems (256B rows - dma_gather granularity)
            T2   = cvec_w @ (R_w@Bc_w).T     (value path, Bc/R folded)
            chat = normalize(cvec_w@Ac_w.T + Ac_b)   (eps-clamped)
  THAT[v] = [ that[v] (60) | 0 (4) ]   bf16
            that = normalize(tvec_w@At_w.T + At_b)

Gathers use the gpsimd dma_gather ucode (max 1024 idxs/instruction,
int16 idxs) - 4 instructions per group, one per 25000-row vocab range,
with idx16 = id - 25000k.  That scrambles ctx order, so attention is
computed densely per group ([128ctx x 128tr] per chunk) with the
row-match + pad-mask folded into the cos matmul as 17 augmentation
rows: cos_aug = cos + sum_b A[b,ctx]*H[b,tr] - 96, where A = 96 *
onehot(row(ctx)) * valid and H = onehot(row(tr)).  Matched+valid pairs
get cos, everything else cos-96 -> exp ~ 0.  Softmax denominator rides
along as T2 column 128 (=64.0): one PSUM region accumulates
[64*s2 | 64*sigma] and the 64s cancel in z = s2/sigma + c2b.
"""

import os
import numpy as np
import ml_dtypes

import concourse.bass as bass
import concourse.bacc as bacc
import concourse.mybir as mybir
import concourse.tile as tile
from concourse.bass_utils import run_bass_kernel_spmd
from concourse.masks import make_identity

F32 = mybir.dt.float32
BF16 = mybir.dt.bfloat16
FP8 = mybir.dt.float8e4
I32 = mybir.dt.int32
I16 = mybir.dt.int16
AF = mybir.ActivationFunctionType
OP = mybir.AluOpType

V, E, DA = 100000, 128, 60
B, T, C = 4096, 8, 200
NCORES = 8
BL = B // NCORES          # 512 local batch rows
NB = 16                   # batch rows per group (NB*T = 128 partitions)
NG = BL // NB             # 32 groups
NR = 4                    # vocab ranges for int16 dma_gather idxs
VR = V // NR              # 25000 rows per range
RCAP = 1024               # gather capacity per (group, range): 8 chunks
NCR_TIGHT = 7             # compute-chunks per range (<=896 real entries)
NCR_MAX = 8               # fallback when a count exceeds 896
TW8 = 256                 # CT8 row elems (fp8, 256B)
THW = 64                  # THAT row elems (bf16)
KA = 16                   # aug rows (row-onehots; -96 rides the exp bias)
KTOT = THW + KA           # 81 contraction rows for cos
BIGM = 96.0               # mask magnitude (exact in bf16)
TSC = 64.0                # T2 fp8 pre-scale (cancels against sigma col)
EPS = 1e-6

_CACHE: dict = {}


def _patch_queue_aware_dma_lanes():
    """Pin each SWDGE queue to its own pair of DMASW sem lanes.

    tile_sem_assignment round-robins Pool-DMA instructions across all 8
    DMASW lanes ignoring queue_num, but each lane's semaphore is locked
    to one ucode queue - crossing is a completion/ordering race.  Queue q
    owning lanes {2q, 2q+1} makes crossing impossible.
    """
    from concourse import tile_sem_assignment as tsa
    from concourse import bass_isa
    if getattr(tsa.TileClockTick, "_qaware_patched", False):
        return
    orig = tsa.TileClockTick._assign_tick

    def _qaware(self, inst):
        if (isinstance(inst, tsa.DMAInst)
                and inst.engine == mybir.EngineType.Pool
                and not isinstance(inst, bass_isa.UserSyncedRemoteDMADescs)):
            qn = getattr(inst, "queue_num", None)
            if qn is None:
                qname = getattr(inst, "queue", "") or ""
                qn = (int(qname[len("qPoolDynamic"):] or 0)
                      if str(qname).startswith("qPoolDynamic") else 0)
            cnt = getattr(self, "_q_lane_cnt", None)
            if cnt is None:
                cnt = self._q_lane_cnt = {}
            i = cnt.get(qn, 0)
            cnt[qn] = i + 1
            self.next_sw_dma_idx = (2 * int(qn) + i % 2) % self.swdge_sem_count
        return orig(self, inst)

    tsa.TileClockTick._assign_tick = _qaware
    tsa.TileClockTick._qaware_patched = True


def _pin_act_table():
    """Pin activations to the natural_log_exp_and_others table (Exp+Copy)."""
    from concourse.hw_specs import get_activation_tables
    keep = "natural_log_exp_and_others"
    orig = get_activation_tables("gen3")
    pinned = {k: (v if k == keep else set()) for k, v in orig.items()}
    bacc.get_activation_tables = lambda arch: pinned


def _build(ncr):
    nchk = NR * ncr       # compute chunks per group
    _pin_act_table()
    _patch_queue_aware_dma_lanes()
    nc = bacc.Bacc(
        "TRN2", target_bir_lowering=False, debug=False, num_devices=NCORES,
        num_swdge_queues=4,
    )
    d = {}
    def din(name, shape, dt):
        d[name] = nc.dram_tensor(name, list(shape), dt, kind="ExternalInput").ap()
    din("ct8", [V, TW8], FP8)
    din("that", [V, THW], BF16)
    din("c2row", [1, E], F32)
    din("idx16", [128, NG * NR * (RCAP // 16)], I16)
    din("aug", [KA, NG * nchk * 128], BF16)
    din("hconst", [KA, 128], BF16)
    din("titg", [NB * T, NG], I32)
    din("gcnt", [1, NG * 5], I32)
    z_dram = nc.dram_tensor("z_out", [BL * T, E], F32, kind="ExternalOutput").ap()

    with tile.TileContext(nc) as tc:
        with (
            tc.tile_pool(name="const", bufs=1) as cp,
            tc.tile_pool(name="gath", bufs=3) as gp,
            tc.tile_pool(name="work", bufs=2) as wp,
            tc.tile_pool(name="psT", bufs=2, space="PSUM") as psT,
            tc.tile_pool(name="psC", bufs=2, space="PSUM") as psC,
            tc.tile_pool(name="psZ", bufs=2, space="PSUM") as psZ,
            tc.tile_pool(name="psG", bufs=1, space="PSUM") as psG,
        ):
            # ---- constants ----
            idb = cp.tile([128, 128], BF16, tag="idb")
            make_identity(nc, idb[:])
            ones_row32 = cp.tile([1, 128], F32, tag="onesr")
            nc.gpsimd.memset(ones_row32[:], 1.0)
            c2row = cp.tile([1, E], F32, tag="c2row")
            nc.sync.dma_start(c2row[:], d["c2row"][:])
            idxsb = cp.tile([128, NG * NR * (RCAP // 16)], I16, tag="idx")
            nc.sync.dma_start(idxsb[:], d["idx16"][:])
            titg = cp.tile([NB * T, NG], I32, tag="titg")
            nc.sync.dma_start(titg[:], d["titg"][:])
            gcnt = cp.tile([1, NG * 5], I32, tag="gcnt")
            nc.sync.dma_start(gcnt[:], d["gcnt"][:])
            cnt_regs = [nc.gpsimd.alloc_register(f"gcnt_r{i}") for i in range(5)]

            # c2b = broadcast of (R_w@Bc_b + R_b) to [128, E]
            ps_c2b = psG.tile([E, E], F32, space="PSUM", tag="tt", bufs=1)
            nc.tensor.matmul(ps_c2b[:], lhsT=ones_row32[:], rhs=c2row[:])
            c2b = cp.tile([E, E], F32, tag="c2b")
            nc.scalar.copy(c2b[:], ps_c2b[:])
            neg96 = cp.tile([128, 1], F32, tag="neg96")
            nc.gpsimd.memset(neg96[:], -BIGM)

            CW = RCAP // 16   # idx16 columns per (group, range)
            NW = nchk // 4    # psum waves of 4 chunks
            # one dma_gather per vocab range; queue-aware DMASW lanes
            # (see _patch_queue_aware_dma_lanes) keep sems queue-pure

            for g in range(NG):
                # ---- gathers (8 gather chunks per range in the tile) ----
                ctg = gp.tile([128, NR * 8, TW8], FP8, tag="ct")
                nc.gpsimd.reg_load(cnt_regs, gcnt[0:1, g * 5:g * 5 + 4])
                nc.gpsimd.reg_load(cnt_regs[4], gcnt[0:1, g * 5 + 4:g * 5 + 5])
                # ranges 0-2 whole on async queues; range 3 split 512+512 so
                # queue 0 (whose desc-gen holds the engine) gets a small share
                GS = [(0, 0, RCAP, 0, 1), (1, 64, RCAP, 8, 2),
                      (2, 128, RCAP, 16, 3), (3, 192, 512, 24, 0),
                      (3, 224, 512, 28, (1, 2, 3)[g % 3])]
                for sl_i, (r, co, ni, ch, q) in enumerate(GS):
                    nc.gpsimd.dma_gather(
                        out_ap=ctg[:, ch:ch + ni // 128, :],
                        in_ap=d["ct8"][r * VR:(r + 1) * VR, :],
                        idxs_ap=idxsb[:, g * NR * CW + co:g * NR * CW + co + ni // 16],
                        num_idxs=ni,
                        num_idxs_reg=cnt_regs[sl_i],
                        elem_size=TW8,
                        queue_num=q,
                    )
                thatg = wp.tile([NB * T, THW], BF16, tag="thg")
                nc.gpsimd.indirect_dma_start(
                    out=thatg[:], out_offset=None, in_=d["that"][:],
                    in_offset=bass.IndirectOffsetOnAxis(ap=titg[:, g:g + 1], axis=0),
                )

                # ---- that^T [81, 128]: rows 0:64 transpose, 64:81 H const ----
                ps_tt = psG.tile([THW, 128], F32, space="PSUM", tag="tt", bufs=1)
                nc.tensor.matmul(ps_tt[:], lhsT=thatg[:], rhs=idb[:])
                thatT = wp.tile([KTOT, 128], BF16, tag="thT")
                nc.scalar.copy(thatT[0:THW, :], ps_tt[:])
                nc.sync.dma_start(thatT[THW:KTOT, :], d["hconst"][:])

                # ---- chat^T [81, nchk*128]: rows 0:64 transposes, 64:81 aug ----
                chatT = wp.tile([KTOT, nchk * 128], BF16, tag="chT")
                nc.scalar.dma_start(
                    chatT[THW:KTOT, :],
                    d["aug"][:, g * nchk * 128:(g + 1) * nchk * 128])
                for w in range(NW):
                    ps_t = psT.tile([THW, 512], F32, space="PSUM", tag="tr")
                    for j4 in range(4):
                        cc = 4 * w + j4
                        gj = 8 * (cc // ncr) + cc % ncr
                        nc.tensor.matmul(
                            ps_t[:, j4 * 128:(j4 + 1) * 128],
                            lhsT=ctg[:, gj, E + 1:E + 1 + THW],
                            rhs=idb[:],
                        )
                    cpy = nc.scalar.copy if w % 4 == 0 else nc.vector.tensor_copy
                    cpy(chatT[0:THW, w * 512:(w + 1) * 512], ps_t[:])

                # ---- cos + exp (masked softmax numerators) ----
                ag = wp.tile([128, nchk * 128], BF16, tag="ag")
                for w in range(NW):
                    ps_c = psC.tile([128, 512], F32, space="PSUM", tag="cos")
                    for j4 in range(4):
                        j = 4 * w + j4
                        nc.tensor.matmul(
                            ps_c[:, j4 * 128:(j4 + 1) * 128],
                            lhsT=chatT[:, j * 128:(j + 1) * 128],
                            rhs=thatT[:],
                        )
                    nc.scalar.activation(
                        ag[:, w * 512:(w + 1) * 512], ps_c[:], AF.Exp,
                        bias=neg96[:])

                # ---- s2 | sigma accumulated tr-major [128, 129] ----
                ps_z = psZ.tile([NB * T, E + 4], F32, space="PSUM", tag="z")
                for cc in range(nchk):
                    gj = 8 * (cc // ncr) + cc % ncr
                    nc.tensor.matmul(
                        ps_z[:, 0:E + 1],
                        lhsT=ag[:, cc * 128:(cc + 1) * 128],
                        rhs=ctg[:, gj, 0:E + 1],
                        start=(cc == 0), stop=(cc == nchk - 1),
                    )
                invS = wp.tile([NB * T, 1], F32, tag="invS")
                nc.vector.reciprocal(invS[:], ps_z[:, E:E + 1])
                zout = wp.tile([NB * T, E], F32, tag="zout")
                nc.vector.scalar_tensor_tensor(
                    out=zout[:], in0=ps_z[:, 0:E], scalar=invS[:], in1=c2b[:],
                    op0=OP.mult, op1=OP.add,
                )
                nc.sync.dma_start(z_dram[g * 128:(g + 1) * 128, :], zout[:])

    nc.compile()
    return nc


def _make_tables(inputs):
    """Host-side weight folding: id-dependent rows -> lookup tables."""
    f32 = np.float32
    bf = ml_dtypes.bfloat16
    f8 = ml_dtypes.float8_e4m3fn
    tvec = np.asarray(inputs["tvec_w"], f32)
    cvec = np.asarray(inputs["cvec_w"], f32)
    Acw = np.asarray(inputs["Ac_w"], f32)
    Acb = np.asarray(inputs["Ac_b"], f32)
    Atw = np.asarray(inputs["At_w"], f32)
    Atb = np.asarray(inputs["At_b"], f32)
    Bcw = np.asarray(inputs["Bc_w"], f32)
    Bcb = np.asarray(inputs["Bc_b"], f32)
    Rw = np.asarray(inputs["R_w"], f32)
    Rb = np.asarray(inputs["R_b"], f32)

    tproj = tvec @ Atw.T + Atb
    tproj /= np.maximum(np.linalg.norm(tproj, axis=1, keepdims=True), EPS)
    that = np.zeros((V, THW), f32)
    that[:, 0:DA] = tproj

    cproj = cvec @ Acw.T + Acb
    cproj /= np.maximum(np.linalg.norm(cproj, axis=1, keepdims=True), EPS)
    ct8 = np.zeros((V, TW8), f32)
    ct8[:, 0:E] = TSC * (cvec @ (Rw @ Bcw).T)
    ct8[:, E] = TSC
    ct8[:, E + 1:E + 1 + DA] = cproj

    c2row = (Rw @ Bcb + Rb).reshape(1, E).astype(f32)

    # H[b, tr] = 1 iff tr belongs to local row b
    h = np.zeros((KA, 128), f32)
    for b in range(NB):
        h[b, b * T:(b + 1) * T] = 1.0
    return ct8.astype(f8), that.astype(bf), c2row, h.astype(bf)


def _wrap_idxs(idx):
    """dma_gather idx layout: i -> (partition i%16, col i//16), x8 replicas."""
    n = idx.size
    w = idx.reshape(n // 16, 16).T
    return np.tile(w, (8, 1))


def _prep_core_inputs(inputs, k, ct8, that, c2row, h, ncr, sim_full=False):
    bf = ml_dtypes.bfloat16
    sl = slice(k * BL, (k + 1) * BL)
    tit = np.ascontiguousarray(
        inputs["batch_titems"][sl].astype(np.int32).reshape(NG, NB * T).T)
    cit = inputs["batch_citems"][sl].astype(np.int64).reshape(NG, NB, C)
    msk = np.asarray(inputs["mask_pad_ids"][sl]).reshape(NG, NB, C)

    nchk = NR * ncr
    idx16 = np.zeros((NG, NR, RCAP), np.int16)
    gcnt = np.full((NG, 5), 1, np.int32)
    aug = np.zeros((KA, NG, nchk * 128), np.float32)
    for g in range(NG):
        rng_ids = cit[g] // VR              # [NB, C] range of each ctx
        for r in range(NR):
            # masked ctx contribute nothing - skip gathering them
            keep = (rng_ids == r) & ~msk[g]
            bs, cs = np.nonzero(keep)
            n = bs.size
            assert n <= 128 * ncr, f"range overflow {n} > {128 * ncr}"
            ids = cit[g, bs, cs] - r * VR
            idx16[g, r, :n] = ids.astype(np.int16)
            if g < 3 or sim_full:
                if r < 3:
                    gcnt[g, r] = RCAP       # ring warm-up: gather id-0 pads
                else:
                    gcnt[g, 3] = 512
                    gcnt[g, 4] = 512
            else:
                idx16[g, r, n:] = -1        # trailing -1s are skipped
                if r < 3:
                    gcnt[g, r] = max(n, 1)
                else:
                    gcnt[g, 3] = max(min(n, 512), 1)
                    gcnt[g, 4] = max(n - 512, 1)
                    if n <= 512:
                        idx16[g, r, 512] = 0
            # compute-chunk col: cc = ncr*r + i//128, partition i%128
            cols = (ncr * r + np.arange(n) // 128) * 128 + np.arange(n) % 128
            aug[bs, g, cols] = BIGM
    idxw = np.concatenate(
        [_wrap_idxs(idx16[g, r]) for g in range(NG) for r in range(NR)], axis=1)
    return {
        "ct8": ct8, "that": that, "c2row": c2row, "hconst": h,
        "idx16": np.ascontiguousarray(idxw),
        "gcnt": np.ascontiguousarray(gcnt.reshape(1, NG * 5)),
        "aug": np.ascontiguousarray(
            aug.reshape(KA, NG * nchk * 128).astype(bf)),
        "titg": tit,
    }


def _install_profile_hook():
    """Dev-only: register the axon NTFF hook missing from this image."""
    import sys
    import types
    try:
        import antenv.axon_hooks  # noqa: F401
        return
    except ImportError:
        pass
    from trn_agent_boot.trn_boot import _ntff_profile_via_ctypes
    hook = _ntff_profile_via_ctypes("/opt/axon/libaxon_pjrt.so")
    mod = types.ModuleType("antenv.axon_hooks")
    mod._hook = hook
    mod.set_axon_ntff_profile_hook = lambda h: setattr(mod, "_hook", h)
    mod.get_axon_ntff_profile_hook = lambda: mod._hook
    sys.modules["antenv.axon_hooks"] = mod
    import antenv
    antenv.axon_hooks = mod


def kernel(**inputs) -> np.ndarray:
    inputs = {k: np.asarray(v) for k, v in inputs.items()}
    cit_all = inputs["batch_citems"].astype(np.int64)
    msk_all = np.asarray(inputs["mask_pad_ids"])
    maxcnt = 0
    for k in range(NCORES):
        cit = cit_all[k * BL:(k + 1) * BL].reshape(NG, NB * C) // VR
        mk = msk_all[k * BL:(k + 1) * BL].reshape(NG, NB * C)
        for g in range(NG):
            maxcnt = max(maxcnt, np.bincount(
                cit[g][~mk[g]], minlength=NR).max())
    ncr = NCR_TIGHT if maxcnt <= 128 * NCR_TIGHT else NCR_MAX
    key = f"nc{ncr}"
    if key not in _CACHE:
        _CACHE[key] = _build(ncr)
    nc = _CACHE[key]
    ct8, that, c2row, h = _make_tables(inputs)
    in_maps = [_prep_core_inputs(inputs, k, ct8, that, c2row, h, ncr)
               for k in range(NCORES)]
    trace = bool(int(os.environ.get("KERNEL_TRACE", "0")))
    kw = {}
    if trace:
        try:
            _install_profile_hook()
            import concourse.bass_utils as _bu
            _bu.upload_artifacts = lambda d: d
            tdir = os.environ.get("KERNEL_TRACE_DIR", "/root/problem/_trace")
            import shutil
            shutil.rmtree(tdir, ignore_errors=True)
            os.makedirs(tdir, exist_ok=True)
            kw["tmpdir"] = tdir
        except Exception as e:  # profiling is best-effort
            print(f"trace setup failed: {e}")
            trace = False
    res = run_bass_kernel_spmd(
        nc, in_maps, list(range(NCORES)), trace=trace, **kw,
    )
    _CACHE["last_result"] = res
    z = np.concatenate(
        [res.results[k]["z_out"].reshape(BL, T, E) for k in range(NCORES)], axis=0
    )
    return z.astype(np.float32)


# revision 28
# speedup vs baseline: 1.2014x; 1.2014x over previous
"""AttentiveItemToVec Trainium2 kernel (8 NeuronCores, batch-parallel).

Strategy: fold every id-dependent quantity into host-precomputed lookup
tables so the device kernel is pure gather + attention:

  CT8[v]  = [ 64*T2[v] (0:128) | 64.0 (128) | chat[v] (129:189) | 0 ]
            fp8e4m3, 256 elems (256B rows - dma_gather granularity)
            T2   = cvec_w @ (R_w@Bc_w).T     (value path, Bc/R folded)
            chat = normalize(cvec_w@Ac_w.T + Ac_b)   (eps-clamped)
  THAT[v] = [ that[v] (60) | 0 (4) ]   bf16
            that = normalize(tvec_w@At_w.T + At_b)

Gathers use the gpsimd dma_gather ucode (max 1024 idxs/instruction,
int16 idxs) - 4 instructions per group, one per 25000-row vocab range,
with idx16 = id - 25000k.  That scrambles ctx order, so attention is
computed densely per group ([128ctx x 128tr] per chunk) with the
row-match + pad-mask folded into the cos matmul as 17 augmentation
rows: cos_aug = cos + sum_b A[b,ctx]*H[b,tr] - 96, where A = 96 *
onehot(row(ctx)) * valid and H = onehot(row(tr)).  Matched+valid pairs
get cos, everything else cos-96 -> exp ~ 0.  Softmax denominator rides
along as T2 column 128 (=64.0): one PSUM region accumulates
[64*s2 | 64*sigma] and the 64s cancel in z = s2/sigma + c2b.
"""

import os
import numpy as np
import ml_dtypes

import concourse.bass as bass
import concourse.bacc as bacc
import concourse.mybir as mybir
import concourse.tile as tile
from concourse.bass_utils import run_bass_kernel_spmd
from concourse.masks import make_identity

F32 = mybir.dt.float32
BF16 = mybir.dt.bfloat16
FP8 = mybir.dt.float8e4
I32 = mybir.dt.int32
I16 = mybir.dt.int16
AF = mybir.ActivationFunctionType
OP = mybir.AluOpType

V, E, DA = 100000, 128, 60
B, T, C = 4096, 8, 200
NCORES = 8
BL = B // NCORES          # 512 local batch rows
NB = 16                   # batch rows per group (NB*T = 128 partitions)
NG = BL // NB             # 32 groups
NR = 4                    # vocab ranges for int16 dma_gather idxs
VR = V // NR              # 25000 rows per range
RCAP = 1024               # gather capacity per (group, range): 8 chunks
NCR_TIGHT = 7             # compute-chunks per range (<=896 real entries)
NCR_MAX = 8               # fallback when a count exceeds 896
TW8 = 256                 # CT8 row elems (fp8, 256B)
THW = 64                  # THAT row elems (bf16)
KA = 16                   # aug rows (row-onehots; -96 rides the exp bias)
KTOT = THW + KA           # 81 contraction rows for cos
BIGM = 96.0               # mask magnitude (exact in bf16)
TSC = 64.0                # T2 fp8 pre-scale (cancels against sigma col)
EPS = 1e-6

_CACHE: dict = {}


def _patch_queue_aware_dma_lanes():
    """Pin each SWDGE queue to its own pair of DMASW sem lanes.

    tile_sem_assignment round-robins Pool-DMA instructions across all 8
    DMASW lanes ignoring queue_num, but each lane's semaphore is locked
    to one ucode queue - crossing is a completion/ordering race.  Queue q
    owning lanes {2q, 2q+1} makes crossing impossible.
    """
    from concourse import tile_sem_assignment as tsa
    from concourse import bass_isa
    if getattr(tsa.TileClockTick, "_qaware_patched", False):
        return
    orig = tsa.TileClockTick._assign_tick

    def _qaware(self, inst):
        if (isinstance(inst, tsa.DMAInst)
                and inst.engine == mybir.EngineType.Pool
                and not isinstance(inst, bass_isa.UserSyncedRemoteDMADescs)):
            qn = getattr(inst, "queue_num", None)
            if qn is None:
                qname = getattr(inst, "queue", "") or ""
                qn = (int(qname[len("qPoolDynamic"):] or 0)
                      if str(qname).startswith("qPoolDynamic") else 0)
            cnt = getattr(self, "_q_lane_cnt", None)
            if cnt is None:
                cnt = self._q_lane_cnt = {}
            i = cnt.get(qn, 0)
            cnt[qn] = i + 1
            self.next_sw_dma_idx = (2 * int(qn) + i % 2) % self.swdge_sem_count
        return orig(self, inst)

    tsa.TileClockTick._assign_tick = _qaware
    tsa.TileClockTick._qaware_patched = True


def _pin_act_table():
    """Pin activations to the natural_log_exp_and_others table (Exp+Copy)."""
    from concourse.hw_specs import get_activation_tables
    keep = "natural_log_exp_and_others"
    orig = get_activation_tables("gen3")
    pinned = {k: (v if k == keep else set()) for k, v in orig.items()}
    bacc.get_activation_tables = lambda arch: pinned


def _build(ncr):
    nchk = NR * ncr       # compute chunks per group
    _pin_act_table()
    _patch_queue_aware_dma_lanes()
    nc = bacc.Bacc(
        "TRN2", target_bir_lowering=False, debug=False, num_devices=NCORES,
        num_swdge_queues=4,
    )
    d = {}
    def din(name, shape, dt):
        d[name] = nc.dram_tensor(name, list(shape), dt, kind="ExternalInput").ap()
    din("ct8", [V, TW8], FP8)
    din("that", [V, THW], BF16)
    din("c2row", [1, E], F32)
    din("idx16", [128, NG * NR * (RCAP // 16)], I16)
    din("aug", [KA, NG * nchk * 128], BF16)
    din("hconst", [KA, 128], BF16)
    din("titg", [NB * T, NG], I32)
    din("gcnt", [1, NG * 5], I32)
    z_dram = nc.dram_tensor("z_out", [BL * T, E], F32, kind="ExternalOutput").ap()

    with tile.TileContext(nc) as tc:
        with (
            tc.tile_pool(name="const", bufs=1) as cp,
            tc.tile_pool(name="gath", bufs=3) as gp,
            tc.tile_pool(name="work", bufs=2) as wp,
            tc.tile_pool(name="psT", bufs=2, space="PSUM") as psT,
            tc.tile_pool(name="psC", bufs=2, space="PSUM") as psC,
            tc.tile_pool(name="psZ", bufs=2, space="PSUM") as psZ,
            tc.tile_pool(name="psG", bufs=1, space="PSUM") as psG,
        ):
            # ---- constants ----
            idb = cp.tile([128, 128], BF16, tag="idb")
            make_identity(nc, idb[:])
            ones_row32 = cp.tile([1, 128], F32, tag="onesr")
            nc.gpsimd.memset(ones_row32[:], 1.0)
            c2row = cp.tile([1, E], F32, tag="c2row")
            nc.sync.dma_start(c2row[:], d["c2row"][:])
            idxsb = cp.tile([128, NG * NR * (RCAP // 16)], I16, tag="idx")
            nc.sync.dma_start(idxsb[:], d["idx16"][:])
            titg = cp.tile([NB * T, NG], I32, tag="titg")
            nc.sync.dma_start(titg[:], d["titg"][:])
            gcnt = cp.tile([1, NG * 5], I32, tag="gcnt")
            nc.sync.dma_start(gcnt[:], d["gcnt"][:])
            cnt_regs = [nc.gpsimd.alloc_register(f"gcnt_r{i}") for i in range(5)]

            # c2b = broadcast of (R_w@Bc_b + R_b) to [128, E]
            ps_c2b = psG.tile([E, E], F32, space="PSUM", tag="tt", bufs=1)
            nc.tensor.matmul(ps_c2b[:], lhsT=ones_row32[:], rhs=c2row[:])
            c2b = cp.tile([E, E], F32, tag="c2b")
            nc.scalar.copy(c2b[:], ps_c2b[:])
            neg96 = cp.tile([128, 1], F32, tag="neg96")
            nc.gpsimd.memset(neg96[:], -BIGM)

            CW = RCAP // 16   # idx16 columns per (group, range)
            NW = nchk // 4    # psum waves of 4 chunks
            # one dma_gather per vocab range; queue-aware DMASW lanes
            # (see _patch_queue_aware_dma_lanes) keep sems queue-pure

            for g in range(NG):
                # ---- gathers (8 gather chunks per range in the tile) ----
                ctg = gp.tile([128, NR * 8, TW8], FP8, tag="ct")
                nc.gpsimd.reg_load(cnt_regs[:4], gcnt[0:1, g * 5:g * 5 + 4])
                for r in range(NR):
                    nc.gpsimd.dma_gather(
                        out_ap=ctg[:, 8 * r:8 * r + 8, :],
                        in_ap=d["ct8"][r * VR:(r + 1) * VR, :],
                        idxs_ap=idxsb[:, (g * NR + r) * CW:(g * NR + r + 1) * CW],
                        num_idxs=RCAP,
                        num_idxs_reg=cnt_regs[r],
                        elem_size=TW8,
                        queue_num=(1, 2, 3, 0)[r],
                    )
                thatg = wp.tile([NB * T, THW], BF16, tag="thg")
                nc.gpsimd.indirect_dma_start(
                    out=thatg[:], out_offset=None, in_=d["that"][:],
                    in_offset=bass.IndirectOffsetOnAxis(ap=titg[:, g:g + 1], axis=0),
                )

                # ---- that^T [81, 128]: rows 0:64 transpose, 64:81 H const ----
                ps_tt = psG.tile([THW, 128], F32, space="PSUM", tag="tt", bufs=1)
                nc.tensor.matmul(ps_tt[:], lhsT=thatg[:], rhs=idb[:])
                thatT = wp.tile([KTOT, 128], BF16, tag="thT")
                nc.scalar.copy(thatT[0:THW, :], ps_tt[:])
                nc.sync.dma_start(thatT[THW:KTOT, :], d["hconst"][:])

                # ---- chat^T [81, nchk*128]: rows 0:64 transposes, 64:81 aug ----
                chatT = wp.tile([KTOT, nchk * 128], BF16, tag="chT")
                nc.scalar.dma_start(
                    chatT[THW:KTOT, :],
                    d["aug"][:, g * nchk * 128:(g + 1) * nchk * 128])
                for w in range(NW):
                    ps_t = psT.tile([THW, 512], F32, space="PSUM", tag="tr")
                    for j4 in range(4):
                        cc = 4 * w + j4
                        gj = 8 * (cc // ncr) + cc % ncr
                        nc.tensor.matmul(
                            ps_t[:, j4 * 128:(j4 + 1) * 128],
                            lhsT=ctg[:, gj, E + 1:E + 1 + THW],
                            rhs=idb[:],
                        )
                    cpy = nc.scalar.copy if w % 4 == 0 else nc.vector.tensor_copy
                    cpy(chatT[0:THW, w * 512:(w + 1) * 512], ps_t[:])

                # ---- cos + exp (masked softmax numerators) ----
                ag = wp.tile([128, nchk * 128], BF16, tag="ag")
                for w in range(NW):
                    ps_c = psC.tile([128, 512], F32, space="PSUM", tag="cos")
                    for j4 in range(4):
                        j = 4 * w + j4
                        nc.tensor.matmul(
                            ps_c[:, j4 * 128:(j4 + 1) * 128],
                            lhsT=chatT[:, j * 128:(j + 1) * 128],
                            rhs=thatT[:],
                        )
                    nc.scalar.activation(
                        ag[:, w * 512:(w + 1) * 512], ps_c[:], AF.Exp,
                        bias=neg96[:])

                # ---- s2 | sigma accumulated tr-major [128, 129] ----
                ps_z = psZ.tile([NB * T, E + 4], F32, space="PSUM", tag="z")
                for cc in range(nchk):
                    gj = 8 * (cc // ncr) + cc % ncr
                    nc.tensor.matmul(
                        ps_z[:, 0:E + 1],
                        lhsT=ag[:, cc * 128:(cc + 1) * 128],
                        rhs=ctg[:, gj, 0:E + 1],
                        start=(cc == 0), stop=(cc == nchk - 1),
                    )
                invS = wp.tile([NB * T, 1], F32, tag="invS")
                nc.vector.reciprocal(invS[:], ps_z[:, E:E + 1])
                zout = wp.tile([NB * T, E], F32, tag="zout")
                nc.vector.scalar_tensor_tensor(
                    out=zout[:], in0=ps_z[:, 0:E], scalar=invS[:], in1=c2b[:],
                    op0=OP.mult, op1=OP.add,
                )
                nc.sync.dma_start(z_dram[g * 128:(g + 1) * 128, :], zout[:])

    nc.compile()
    return nc


def _make_tables(inputs):
    """Host-side weight folding: id-dependent rows -> lookup tables."""
    f32 = np.float32
    bf = ml_dtypes.bfloat16
    f8 = ml_dtypes.float8_e4m3fn
    tvec = np.asarray(inputs["tvec_w"], f32)
    cvec = np.asarray(inputs["cvec_w"], f32)
    Acw = np.asarray(inputs["Ac_w"], f32)
    Acb = np.asarray(inputs["Ac_b"], f32)
    Atw = np.asarray(inputs["At_w"], f32)
    Atb = np.asarray(inputs["At_b"], f32)
    Bcw = np.asarray(inputs["Bc_w"], f32)
    Bcb = np.asarray(inputs["Bc_b"], f32)
    Rw = np.asarray(inputs["R_w"], f32)
    Rb = np.asarray(inputs["R_b"], f32)

    tproj = tvec @ Atw.T + Atb
    tproj /= np.maximum(np.linalg.norm(tproj, axis=1, keepdims=True), EPS)
    that = np.zeros((V, THW), f32)
    that[:, 0:DA] = tproj

    cproj = cvec @ Acw.T + Acb
    cproj /= np.maximum(np.linalg.norm(cproj, axis=1, keepdims=True), EPS)
    ct8 = np.zeros((V, TW8), f32)
    ct8[:, 0:E] = TSC * (cvec @ (Rw @ Bcw).T)
    ct8[:, E] = TSC
    ct8[:, E + 1:E + 1 + DA] = cproj

    c2row = (Rw @ Bcb + Rb).reshape(1, E).astype(f32)

    # H[b, tr] = 1 iff tr belongs to local row b
    h = np.zeros((KA, 128), f32)
    for b in range(NB):
        h[b, b * T:(b + 1) * T] = 1.0
    return ct8.astype(f8), that.astype(bf), c2row, h.astype(bf)


def _wrap_idxs(idx):
    """dma_gather idx layout: i -> (partition i%16, col i//16), x8 replicas."""
    n = idx.size
    w = idx.reshape(n // 16, 16).T
    return np.tile(w, (8, 1))


def _prep_core_inputs(inputs, k, ct8, that, c2row, h, ncr, sim_full=False):
    bf = ml_dtypes.bfloat16
    sl = slice(k * BL, (k + 1) * BL)
    tit = np.ascontiguousarray(
        inputs["batch_titems"][sl].astype(np.int32).reshape(NG, NB * T).T)
    cit = inputs["batch_citems"][sl].astype(np.int64).reshape(NG, NB, C)
    msk = np.asarray(inputs["mask_pad_ids"][sl]).reshape(NG, NB, C)

    nchk = NR * ncr
    idx16 = np.zeros((NG, NR, RCAP), np.int16)
    gcnt = np.full((NG, 5), 1, np.int32)
    aug = np.zeros((KA, NG, nchk * 128), np.float32)
    for g in range(NG):
        rng_ids = cit[g] // VR              # [NB, C] range of each ctx
        for r in range(NR):
            # masked ctx contribute nothing - skip gathering them
            keep = (rng_ids == r) & ~msk[g]
            bs, cs = np.nonzero(keep)
            n = bs.size
            assert n <= 128 * ncr, f"range overflow {n} > {128 * ncr}"
            ids = cit[g, bs, cs] - r * VR
            idx16[g, r, :n] = ids.astype(np.int16)
            if g < 3 or sim_full:
                gcnt[g, r] = RCAP           # ring warm-up: gather id-0 pads
            else:
                idx16[g, r, n:] = -1        # trailing -1s are skipped
                gcnt[g, r] = max(n, 1)
            # compute-chunk col: cc = ncr*r + i//128, partition i%128
            cols = (ncr * r + np.arange(n) // 128) * 128 + np.arange(n) % 128
            aug[bs, g, cols] = BIGM
    idxw = np.concatenate(
        [_wrap_idxs(idx16[g, r]) for g in range(NG) for r in range(NR)], axis=1)
    return {
        "ct8": ct8, "that": that, "c2row": c2row, "hconst": h,
        "idx16": np.ascontiguousarray(idxw),
        "gcnt": np.ascontiguousarray(gcnt.reshape(1, NG * 5)),
        "aug": np.ascontiguousarray(
            aug.reshape(KA, NG * nchk * 128).astype(bf)),
        "titg": tit,
    }


def _install_profile_hook():
    """Dev-only: register the axon NTFF hook missing from this image."""
    import sys
    import types
    try:
        import antenv.axon_hooks  # noqa: F401
        return
    except ImportError:
        pass
    from trn_agent_boot.trn_boot import _ntff_profile_via_ctypes
    hook = _ntff_profile_via_ctypes("/opt/axon/libaxon_pjrt.so")
    mod = types.ModuleType("antenv.axon_hooks")
    mod._hook = hook
    mod.set_axon_ntff_profile_hook = lambda h: setattr(mod, "_hook", h)
    mod.get_axon_ntff_profile_hook = lambda: mod._hook
    sys.modules["antenv.axon_hooks"] = mod
    import antenv
    antenv.axon_hooks = mod


def kernel(**inputs) -> np.ndarray:
    inputs = {k: np.asarray(v) for k, v in inputs.items()}
    cit_all = inputs["batch_citems"].astype(np.int64)
    msk_all = np.asarray(inputs["mask_pad_ids"])
    maxcnt = 0
    for k in range(NCORES):
        cit = cit_all[k * BL:(k + 1) * BL].reshape(NG, NB * C) // VR
        mk = msk_all[k * BL:(k + 1) * BL].reshape(NG, NB * C)
        for g in range(NG):
            maxcnt = max(maxcnt, np.bincount(
                cit[g][~mk[g]], minlength=NR).max())
    ncr = NCR_TIGHT if maxcnt <= 128 * NCR_TIGHT else NCR_MAX
    key = f"nc{ncr}"
    if key not in _CACHE:
        _CACHE[key] = _build(ncr)
    nc = _CACHE[key]
    ct8, that, c2row, h = _make_tables(inputs)
    in_maps = [_prep_core_inputs(inputs, k, ct8, that, c2row, h, ncr)
               for k in range(NCORES)]
    trace = bool(int(os.environ.get("KERNEL_TRACE", "0")))
    kw = {}
    if trace:
        try:
            _install_profile_hook()
            import concourse.bass_utils as _bu
            _bu.upload_artifacts = lambda d: d
            tdir = os.environ.get("KERNEL_TRACE_DIR", "/root/problem/_trace")
            import shutil
            shutil.rmtree(tdir, ignore_errors=True)
            os.makedirs(tdir, exist_ok=True)
            kw["tmpdir"] = tdir
        except Exception as e:  # profiling is best-effort
            print(f"trace setup failed: {e}")
            trace = False
    res = run_bass_kernel_spmd(
        nc, in_maps, list(range(NCORES)), trace=trace, **kw,
    )
    _CACHE["last_result"] = res
    z = np.concatenate(
        [res.results[k]["z_out"].reshape(BL, T, E) for k in range(NCORES)], axis=0
    )
    return z.astype(np.float32)
